# revision 28
# baseline (speedup 1.0000x reference)
"""Box-from-mask kernel for Trainium2 (8 NeuronCores, SPMD data-parallel).

Problem: masks [100, 800, 1280] f32 -> boxes [100, 2, 2] f32 where
box[n] = [[xmin, ymin], [xmax, ymax]] of {(y, x) : masks[n, y, x] > 0.5},
with empty-mask sentinels xmin=W, ymin=H, xmax=-1, ymax=-1.

Sharding: the flattened row axis (100*800 = 80000 rows of 1280 px) splits
into 8 contiguous shards of 10000 rows (= 25 half-mask "units" of 400 rows
each). Each core streams its shard once; the stream is DMA/HBM-bound, so
everything else hides under it and the post-stream drain is kept short.

Tiles are 128 rows (78 tiles + a 16-row runt). Stream DMAs fetch S tiles
interleaved - partition p holds tile rows {i*128+p} - keeping one 5120 B
descriptor per row; measured DMA-engine behavior: a contiguous 2D fetch
merges into one descriptor on ONE engine, and any partition count other
than 128 takes a slow non-swizzle split (125 -> 5 engines; 120 -> 15
engines at half rate), so full-128 tiles are the only fast shape. All
DMAs ride HWDGE rings (SP: mask stream + final outputs; ACT: consts +
early column result); SWDGE is unused. Chunk sizes taper (6,...,3,2,1)
so the final pipeline drain is one small tile.

Per 125-row tile: one elementwise pass (DVE is_gt / ACT relu alternating)
yields a 0/1-ish bf16 tile whose accum_out gives per-row "any"; PE matmul
with a 25-column routing weight (sliding one-hot window; custom windows
for the 20 unit-boundary tiles) accumulates per-column counts per unit.
Column counts close in two PSUM generations: tiles 0-77 stop while the
stream still runs (their (count>0) pass and 64 KB DMA hide under it);
tiles 78-79 (unit 24 only, [1, W] PSUM) resolve in the drain, with the
last tile's elementwise pass split into DVE || ACT halves. The host
assembles exact min/max box coordinates from ~46 KB/core of bitmaps.
"""

import sys

for _p in ("/opt/trn_rl_repo", "/opt/pypackages"):
    if _p not in sys.path:
        sys.path.append(_p)

import ml_dtypes
import numpy as np

import concourse.bass as bass
import concourse.tile as tile
from concourse import bacc, mybir
from concourse.bass_utils import run_bass_kernel_spmd

N, H, W = 100, 800, 1280
N_CORES = 8
THRESHOLD = 0.5

HU = 400  # rows per unit (half mask)
K = 25  # units per core
R = K * HU  # rows per core (10000)
P = 128  # rows per tile: only the full-128 outer dim takes the DMA
# engines' fast swizzle-aligned 16-way split (120 -> 15 engines at half
# rate each; 125 -> 5; contiguous merge -> 1)
NT = 78  # full tiles (9984 rows)
NA = 72  # tiles 0..NA-1 -> main PSUM generation (rows 0-9215)
RUNT = R - P * NT  # 16 trailing rows
CHUNKS = [6] * 11 + [3, 3, 2, 2, 1, 1]  # tiles per stream DMA (sum = 78)
MAXS = max(CHUNKS)
HALF = W // 2

fp32 = mybir.dt.float32
fp16 = mybir.dt.float16
bf16 = mybir.dt.bfloat16
Op = mybir.AluOpType

_chunk_cols = [(0, 512), (512, 512), (1024, 256)]


def _tile_units(t):
    """(first_unit, n_units) covered by rows [P*t, P*t+P)."""
    u0 = (P * t) // HU
    u1 = (P * t + P - 1) // HU
    return u0, u1 - u0 + 1


_boundary = [t for t in range(NA) if _tile_units(t)[1] > 1]
# weight const layout: [one-hot bank 2K cols][custom windows K cols each]
_OH = 2 * K
_cust_off = {t: _OH + K * i for i, t in enumerate(_boundary)}
WCOLS = _OH + K * len(_boundary)


def _wslice(t):
    """(col0, width) of tile t's routing window in the weight const."""
    if t < NA:
        if t in _cust_off:
            return _cust_off[t], K
        return K - 1 - _tile_units(t)[0], K  # one-hot at local col u0
    # taper tiles 72-77 are single-unit (unit 23 or 24) -> [2, cw] psum
    return K - 1 - (_tile_units(t)[0] - (K - 2)), 2


def build_program():
    """One-core Bass/Tile program; run SPMD on all 8 cores."""
    nc = bacc.Bacc(
        "TRN2", target_bir_lowering=False, debug=False, enable_asserts=False
    )
    masks = nc.dram_tensor("masks", [R, W], fp32, kind="ExternalInput").ap()
    wmat = nc.dram_tensor("wmat", [128, WCOLS], bf16, kind="ExternalInput").ap()
    rowany_out = nc.dram_tensor(
        "rowany_out", [128, NT + 1], fp32, kind="ExternalOutput"
    ).ap()
    colany_out = nc.dram_tensor("colany_out", [K, W], fp16, kind="ExternalOutput").ap()
    colb16_out = nc.dram_tensor("colb16_out", [2, 768], fp16, kind="ExternalOutput").ap()
    colb32_out = nc.dram_tensor("colb32_out", [2, 512], fp32, kind="ExternalOutput").ap()
    runt_raw = nc.dram_tensor("runt_raw", [RUNT, W], bf16, kind="ExternalOutput").ap()
    t76_raw = nc.dram_tensor("t76_raw", [P, W], bf16, kind="ExternalOutput").ap()
    t77_raw = nc.dram_tensor("t77_raw", [P, W], bf16, kind="ExternalOutput").ap()

    with tile.TileContext(nc) as tc:
        with (
            tc.tile_pool(name="raw", bufs=4) as rawp,
            tc.tile_pool(name="bin", bufs=12) as binp,
            tc.tile_pool(name="consts", bufs=1) as constp,
            tc.tile_pool(name="psum", bufs=1, space="PSUM") as psump,
        ):
            # consts on the ACT HWDGE ring; the SP ring belongs to the
            # mask stream, and SWDGE stays entirely unused
            wmat_t = constp.tile([128, WCOLS], bf16)
            nc.scalar.dma_start(wmat_t[:], wmat)
            rowany = constp.tile([128, NT + 1], fp32)
            nc.gpsimd.memset(rowany[:], 0.0)
            negh = constp.tile([128, 1], fp32)
            nc.gpsimd.memset(negh[:], -THRESHOLD)
            cola_sb = constp.tile([K, W], fp16)
            colb16_sb = constp.tile([2, 768], fp16)
            colb32_sb = constp.tile([2, 512], fp32)
            cca = [
                psump.tile([K, cw], fp32, name=f"cca{ci}", tag=f"cca{ci}")
                for ci, (_, cw) in enumerate(_chunk_cols)
            ]
            ccb = [
                psump.tile([2, cw], fp32, name=f"ccb{ci}", tag=f"ccb{ci}")
                for ci, (_, cw) in enumerate(_chunk_cols)
            ]

            eng_flip = [0]

            def binarize(out_b, rv, acc, nr, force=None):
                """One elementwise pass: binary tile for PE + row-any accum."""
                eng = force if force else ("dve" if eng_flip[0] % 2 == 0 else "act")
                if eng == "dve":
                    if acc is None:
                        nc.vector.tensor_scalar(
                            out=out_b, in0=rv, scalar1=THRESHOLD,
                            scalar2=None, op0=Op.is_gt,
                        )
                    else:
                        nc.vector.tensor_scalar(
                            out=out_b,
                            in0=rv,
                            scalar1=THRESHOLD,
                            scalar2=None,
                            op0=Op.is_gt,
                            op1=Op.max,
                            accum_out=acc,
                        )
                elif acc is None:
                    nc.scalar.activation(
                        out=out_b,
                        in_=rv,
                        func=mybir.ActivationFunctionType.Relu,
                        bias=negh[:nr, :],
                        scale=1.0,
                    )
                else:
                    nc.scalar.activation(
                        out=out_b,
                        in_=rv,
                        func=mybir.ActivationFunctionType.Relu,
                        bias=negh[:nr, :],
                        scale=1.0,
                        accum_out=acc,
                    )
                if force is None:
                    eng_flip[0] += 1

            t = 0
            base = 0
            ship = {}
            for S in CHUNKS:
                raw = rawp.tile([128, MAXS * W], fp32, tag="raw")
                nc.sync.dma_start(
                    raw[:P, : S * W],
                    masks[base : base + P * S, :].rearrange(
                        "(a p) x -> p a x", p=P
                    ),
                )
                for i in range(S):
                    b = binp.tile([128, W], bf16, tag="b")
                    rv = raw[:P, i * W : (i + 1) * W]
                    if t == NT - 1:
                        # last tile: halves on DVE || ACT to shorten drain
                        binarize(
                            b[:P, :HALF], rv[:, :HALF],
                            rowany[:P, t : t + 1], P, force="dve",
                        )
                        binarize(
                            b[:P, HALF:], rv[:, HALF:],
                            rowany[:P, NT : NT + 1], P, force="act",
                        )
                    else:
                        binarize(b[:P, :], rv, rowany[:P, t : t + 1], P)
                    if t >= NT - 2:
                        # tiles 76/77 (unit 24 only) skip PE: their
                        # binarized tiles ship raw and the host ORs them
                        ship[t] = b
                    else:
                        cc = cca if t < NA else ccb
                        w0, wn = _wslice(t)
                        for ci, (c0, cw) in enumerate(_chunk_cols):
                            nc.tensor.matmul(
                                cc[ci][:, :],
                                wmat_t[:P, w0 : w0 + wn],
                                b[:P, c0 : c0 + cw],
                                start=(t == 0 or t == NA),
                                stop=(t == NA - 1 or t == NT - 3),
                            )
                    t += 1

                base += P * S

            # trailing 16 rows, fetched as two strided x-half DMAs (a
            # contiguous [16, W] fetch would merge into one descriptor on
            # one engine); halves binarize on DVE || ACT and the binarized
            # tile ships raw - no PE on the runt path
            raw = rawp.tile([128, MAXS * W], fp32, tag="raw")
            nc.sync.dma_start(raw[:RUNT, :HALF], masks[P * NT : R, :HALF])
            nc.sync.dma_start(raw[:RUNT, HALF:W], masks[P * NT : R, HALF:])
            br = binp.tile([128, W], bf16, tag="b")
            binarize(br[:RUNT, :HALF], raw[:RUNT, :HALF], None, RUNT, force="dve")
            binarize(br[:RUNT, HALF:], raw[:RUNT, HALF:W], None, RUNT, force="act")
            # main-generation (count>0) sits here so it fills DVE's
            # wait-for-PE gap instead of stalling the taper binarizes
            for ci, (c0, cw) in enumerate(_chunk_cols):
                nc.vector.tensor_scalar(
                    out=cola_sb[:, c0 : c0 + cw],
                    in0=cca[ci][:, :],
                    scalar1=0.0,
                    scalar2=None,
                    op0=Op.is_gt,
                )
            # taper-generation (count>0): chunks 0/2 on DVE, chunk 1 on ACT
            # (relu keeps positives positive; f32 out so tiny relu-sums
            # cannot round to zero)
            nc.vector.tensor_scalar(
                out=colb16_sb[:, 0:512],
                in0=ccb[0][:, :],
                scalar1=0.0,
                scalar2=None,
                op0=Op.is_gt,
            )
            nc.scalar.activation(
                out=colb32_sb[:],
                in_=ccb[1][:, :],
                func=mybir.ActivationFunctionType.Relu,
                bias=0.0,
                scale=1.0,
            )
            nc.vector.tensor_scalar(
                out=colb16_sb[:, 512:768],
                in0=ccb[2][:, :],
                scalar1=0.0,
                scalar2=None,
                op0=Op.is_gt,
            )
            # outputs split across both idle HWDGE rings, in readiness
            # order: colany is done before the stream ends; rowany only
            # needs the last tile's halves (not the runt)
            nc.sync.dma_start(colany_out, cola_sb[:])
            nc.scalar.dma_start(rowany_out, rowany[:])
            nc.sync.dma_start(t76_raw, ship[NT - 2][:P, :])
            nc.scalar.dma_start(t77_raw, ship[NT - 1][:P, :])
            nc.sync.dma_start(colb32_out, colb32_sb[:])
            nc.scalar.dma_start(runt_raw, br[:RUNT, :])
            nc.sync.dma_start(colb16_out, colb16_sb[:])

    nc.compile()
    return nc


def make_wmat():
    """Routing weights: sliding one-hot bank + boundary-tile windows."""
    wmat = np.zeros((128, WCOLS), ml_dtypes.bfloat16)
    p = np.arange(P)
    wmat[:, K - 1] = 1  # one-hot bank: col K-1
    for t, off in _cust_off.items():
        units = (P * t + p) // HU
        wmat[p, off + units] = 1
    return wmat


_cache = {}


def _get_program():
    if "nc" not in _cache:
        _cache["nc"] = build_program()
        _cache["wmat"] = make_wmat()
    return _cache["nc"], _cache["wmat"]


def make_in_maps(masks):
    masks = np.ascontiguousarray(np.asarray(masks, dtype=np.float32))
    _, wmat = _get_program()
    rows = masks.reshape(N_CORES, R, W)
    return [{"masks": rows[c], "wmat": wmat} for c in range(N_CORES)]


def postprocess(results):
    """Per-core any-bitmaps -> boxes [N, 2, 2] f32 (exact integer math)."""
    nu = N_CORES * K  # 200 units (half masks)
    u_ymin = np.full(nu, float(H))
    u_ymax = np.full(nu, -1.0)
    u_xmin = np.full(nu, float(W))
    u_xmax = np.full(nu, -1.0)
    ys = np.arange(HU)
    xs = np.arange(W)
    for c, r in enumerate(results):
        ra = np.asarray(r["rowany_out"], np.float32) > 0  # [128, NT+1]
        ra[:, NT - 1] |= ra[:, NT]  # tile NT-1 = DVE half | ACT half
        rows_any = np.empty(R, bool)
        s = 0
        base = 0
        for S in CHUNKS:
            blk = ra[:P, s : s + S]  # [P, S]; row = base + P*i + p
            rows_any[base : base + P * S] = blk.T.reshape(-1)
            s += S
            base += P * S
        ca = np.asarray(r["colany_out"], np.float32) > 0  # [K, W] units 0-24
        b16 = np.asarray(r["colb16_out"], np.float32) > 0  # [2, 768]
        b32 = np.asarray(r["colb32_out"], np.float32) > 0  # [2, 512]
        rr = np.asarray(r["runt_raw"], np.float32) > 0  # [RUNT, W]
        rows_any[P * NT :] = rr.any(1)
        for j in range(2):
            ca[K - 2 + j] |= np.concatenate(
                [b16[j, 0:512], b32[j], b16[j, 512:768]]
            )
        ca[K - 1] |= rr.any(0)
        ca[K - 1] |= (np.asarray(r["t76_raw"], np.float32) > 0).any(0)
        ca[K - 1] |= (np.asarray(r["t77_raw"], np.float32) > 0).any(0)

        A = rows_any.reshape(K, HU)
        g = c * K + np.arange(K)
        off = (g % 2) * HU  # row offset of this unit within its mask
        has = A.any(1)
        u_ymin[g] = np.where(has, off + np.where(A, ys, H).min(1), H)
        u_ymax[g] = np.where(has, off + np.where(A, ys, -1).max(1), -1)
        hasx = ca.any(1)
        u_xmin[g] = np.where(hasx, np.where(ca, xs, W).min(1), W)
        u_xmax[g] = np.where(hasx, np.where(ca, xs, -1).max(1), -1)

    boxes = np.empty((N, 2, 2), np.float32)
    boxes[:, 0, 0] = u_xmin.reshape(N, 2).min(1)
    boxes[:, 0, 1] = u_ymin.reshape(N, 2).min(1)
    boxes[:, 1, 0] = u_xmax.reshape(N, 2).max(1)
    boxes[:, 1, 1] = u_ymax.reshape(N, 2).max(1)
    return boxes


def kernel(masks):
    nc, _ = _get_program()
    in_maps = make_in_maps(masks)
    res = run_bass_kernel_spmd(nc, in_maps, core_ids=list(range(N_CORES)))
    return postprocess(res.results)
